# revision 2
# baseline (speedup 1.0000x reference)
"""Banded (Longformer-style) multi-head attention on 8 TRN2 NeuronCores.

Sharding: 16 heads are split 2-per-core (tensor parallel on H); every
core sees all 8192 tokens.  Compute dtype is bf16 (f32 accumulate in
PSUM); inputs are pre-cast/pre-TRANSPOSED on the host, so every device
DMA is a plain strided copy (no descriptor-heavy DMA transposes).

Per-core kernel (single NEFF, software-pipelined emission so proj /
attention / out-proj overlap):
  1. DMA feature-major xT slabs DRAM->SBUF; project to qT,kT
     (feature-major [d, T]) via w-stationary matmuls and to v TOKEN-major
     via x-stationary matmuls (same FLOPs, kills the v transpose).  v is
     stored ones-augmented so the P@V matmul also produces the softmax
     denominator.  The v bias is folded into the output bias on the host
     (exact: attention rows sum to 1).
  2. j-major banded attention: for each 128-wide key tile j, one
     scores^T matmul [key,y x query-cols] against the <=5 query chunks
     in its band (K=64), exp on ScalarE without max-subtraction
     (scores are O(+-30), exact in f32), band-corner masking via
     affine_select on the GpSimd engine, then per-query-chunk
     P^T@V_aug accumulation (K=128) and a 1/den fixup on VectorE.
  3. ctx chunks transposed feature-major ON THE PE (128-cycle transpose
     through an identity ldweights) instead of DMA; partial output
     projection ctx_h @ Wo_h.T -> [8192, 1024] bf16.
The host sums the 8 partial outputs and adds the output bias (the
all-reduce step of tensor parallelism, done during the gather).
"""

import sys

sys.path.insert(0, "/opt/trn_rl_repo")

import numpy as np

import concourse.bass as bass
import concourse.mybir as mybir
import concourse.tile as tile
from concourse import bacc
from concourse import masks
from concourse.bass_utils import run_bass_kernel_spmd

F32 = mybir.dt.float32
BF16 = mybir.dt.bfloat16

B, S, D, E, H, HD = 2, 4096, 1024, 1024, 16, 64
W = 256                    # half window
T = B * S                  # 8192 flattened tokens
NCORES = 8
HPC = H // NCORES          # 2 heads per core
FQKV = 3 * HPC * HD        # 384 projected features per core
NT = T // 128              # 64 token chunks
CPS = S // 128             # 32 chunks per sequence
SLAB = 512                 # proj token slab
NSLAB = T // SLAB          # 16
VROW = 2 * (HD + 1)        # 130: [v_h0(64) | 1 | v_h1(64) | 1]


def _build_program():
    nc = bacc.Bacc(None, target_bir_lowering=False, debug=False)

    xT_d = nc.dram_tensor("xbfT", [D, T], BF16, kind="ExternalInput")
    wqkvT_d = nc.dram_tensor("wqkvT", [D, FQKV], BF16, kind="ExternalInput")
    bqkv_d = nc.dram_tensor("bqkv", [FQKV], F32, kind="ExternalInput")
    woT_d = nc.dram_tensor("woT", [HPC * HD, E], BF16, kind="ExternalInput")
    out_d = nc.dram_tensor("out_p", [T, E], BF16, kind="ExternalOutput")

    with tile.TileContext(nc) as tc:
        with (
            tc.tile_pool(name="const", bufs=1) as cpool,
            tc.tile_pool(name="big", bufs=1) as bigpool,
            tc.tile_pool(name="xtp", bufs=3) as xtp,
            tc.tile_pool(name="cnp", bufs=4) as cnp,
            tc.tile_pool(name="recp", bufs=4) as recp,
            tc.tile_pool(name="ptp", bufs=8) as ptp,
            tc.tile_pool(name="outsb", bufs=2) as outsb,
            tc.tile_pool(name="ps512", bufs=2, space="PSUM") as ps512,
            tc.tile_pool(name="spsum", bufs=2, space="PSUM") as spsum,
            tc.tile_pool(name="cpsum", bufs=1, space="PSUM") as cpsum,
            tc.tile_pool(name="tpsum", bufs=1, space="PSUM") as tpsum,
        ):
            # ---- constants ----
            w_sb = cpool.tile([128, 8, FQKV], BF16, tag="w_sb")
            nc.sync.dma_start(
                w_sb[:], wqkvT_d[:].rearrange("(c p) f -> p c f", p=128))
            wo_sb = cpool.tile([128, E], BF16, tag="wo_sb")
            nc.sync.dma_start(wo_sb[:], woT_d[:])
            b_sb = cpool.tile([128, 3], F32, tag="b_sb")
            nc.sync.dma_start(b_sb[:], bqkv_d[:].rearrange("(a p) -> p a", p=128))
            ident = cpool.tile([128, 128], BF16, tag="ident")
            masks.make_identity(nc, ident[:])

            # ---- persistent activations ----
            q_sb = bigpool.tile([128, T], BF16, tag="q_sb")
            k_sb = bigpool.tile([128, T], BF16, tag="k_sb")
            v_sb = bigpool.tile([128, NT, VROW], BF16, tag="v_sb")
            ctxT_sb = bigpool.tile([128, T], BF16, tag="ctxT_sb")
            # ones columns of the augmented V (cols 64 and 129 of each chunk)
            nc.vector.memset(v_sb[:, :, HD::HD + 1], 1.0)

            # manually-rotated PSUM slot tiles (each fits in one 2KB bank)
            ctx_ps = cpsum.tile([128, 4, HD + 1], F32, tag="ctx_ps")
            ctxT_ps = tpsum.tile([128, 4, 128], BF16, tag="ctxT_ps")

            def proj_slab(t0, ntok):
                # ntok tokens starting at t0 (multiple of 128, <= 512)
                nck = ntok // 128
                xT = xtp.tile([128, 8, SLAB], BF16, tag="xT")
                nc.sync.dma_start(
                    xT[:, :, 0:ntok],
                    xT_d[:, t0:t0 + ntok].rearrange("(c p) t -> p c t", p=128))
                # q, k: feature-major [128 feats, ntok]
                for ft in range(2):
                    ps = ps512.tile([128, SLAB], F32, tag="ps512")
                    for c in range(8):
                        nc.tensor.matmul(
                            ps[:, 0:ntok], w_sb[:, c, ft * 128:(ft + 1) * 128],
                            xT[:, c, 0:ntok], start=(c == 0), stop=(c == 7))
                    dest = (q_sb, k_sb)[ft]
                    nc.vector.tensor_scalar_add(
                        dest[:, t0:t0 + ntok], ps[:, 0:ntok], b_sb[:, ft:ft + 1])
                # v: token-major [128 tokens, 128 feats] per chunk (no
                # transpose needed; v bias is folded into bo on the host)
                vps = ps512.tile([128, SLAB], F32, tag="ps512")
                for ck in range(nck):
                    for c in range(8):
                        nc.tensor.matmul(
                            vps[:, ck * 128:(ck + 1) * 128],
                            xT[:, c, ck * 128:(ck + 1) * 128],
                            w_sb[:, c, 2 * 128:3 * 128],
                            start=(c == 0), stop=(c == 7))
                for ck in range(nck):
                    gck = t0 // 128 + ck
                    nc.vector.tensor_copy(
                        v_sb[:, gck, :].rearrange("p (h r) -> p h r", h=2)[:, :, 0:HD],
                        vps[:, ck * 128:(ck + 1) * 128].rearrange(
                            "p (h r) -> p h r", h=2))

            # j-major scoresT: st_j[y, b*128:(b+1)*128] = k_j^T q_{c}, where
            # c = j-2+b.  pt_j = exp(st_j/8) with band corners zeroed via
            # affine_select on GpSimd.
            pt_tiles = {}
            cn_tiles = {}

            def scores_j(seq, j, h):
                b_lo = max(0, 2 - j)
                b_hi = min(4, 2 + (CPS - 1) - j)
                gj = seq * CPS + j
                st = spsum.tile([128, 640], F32, tag="st")
                lo, hi = b_lo * 128, (b_hi + 1) * 128
                qcols = (seq * CPS + j - 2) * 128
                pieces = [(a, b) for (a, b) in [(lo, min(hi, 512)), (512, hi)]
                          if b > a]
                for (a, b) in pieces:
                    nc.tensor.matmul(
                        st[:, a:b],
                        k_sb[h * HD:(h + 1) * HD, gj * 128:(gj + 1) * 128],
                        q_sb[h * HD:(h + 1) * HD, qcols + a:qcols + b],
                        start=True, stop=True)
                pt = ptp.tile([128, 640], BF16, tag="pt")
                nc.scalar.activation(
                    pt[:, lo:hi], st[:, lo:hi],
                    mybir.ActivationFunctionType.Exp,
                    scale=float(1.0 / np.sqrt(HD)))
                if b_lo == 0:
                    # b=0 <-> chunk c=j-2, m=4: keep y <= t  (p <= f)
                    nc.gpsimd.affine_select(
                        out=pt[:, 0:128], in_=pt[:, 0:128],
                        compare_op=mybir.AluOpType.is_ge, fill=0.0, base=0,
                        pattern=[[1, 128]], channel_multiplier=-1)
                if b_hi == 4:
                    # b=4 <-> chunk c=j+2, m=0: keep y >= t  (p >= f)
                    nc.gpsimd.affine_select(
                        out=pt[:, 512:640], in_=pt[:, 512:640],
                        compare_op=mybir.AluOpType.is_ge, fill=0.0, base=0,
                        pattern=[[-1, 128]], channel_multiplier=1)
                pt_tiles[(seq, j, h)] = pt

            def attention_chunk(gc):
                seq, c = divmod(gc, CPS)
                m_lo = max(0, 2 - c)
                m_hi = min(4, CPS - 1 - c + 2)
                nm = m_hi - m_lo + 1
                cn = cnp.tile([128, 128], BF16, tag="cn")
                cn_tiles[gc] = cn
                for h in range(HPC):
                    slot = (gc % 2) * 2 + h
                    ctx = ctx_ps[:, slot, :]
                    for mi, m in enumerate(range(m_lo, m_hi + 1)):
                        j = c - 2 + m
                        pt = pt_tiles[(seq, j, h)]
                        b = c - j + 2
                        nc.tensor.matmul(
                            ctx, pt[:, b * 128:(b + 1) * 128],
                            v_sb[:, seq * CPS + j,
                                 h * (HD + 1):(h + 1) * (HD + 1)],
                            start=(mi == 0), stop=(mi == nm - 1))
                    rec = recp.tile([128, 1], F32, tag="rec")
                    nc.vector.reciprocal(rec[:], ctx[:, HD:HD + 1])
                    nc.vector.tensor_scalar_mul(cn[:, h * HD:(h + 1) * HD],
                                                ctx[:, 0:HD], rec[:])

            def transpose_chunk(gc):
                # ctx chunk token-major -> feature-major via the PE
                cn = cn_tiles.pop(gc)
                tslot = gc % 4
                nc.tensor.transpose(ctxT_ps[:, tslot, :], cn[:], ident[:])
                nc.vector.tensor_copy(ctxT_sb[:, gc * 128:(gc + 1) * 128],
                                      ctxT_ps[:, tslot, :])

            def outproj_quad(qi):
                ob = outsb.tile([128, 4, E], BF16, tag="ob")
                for ci in range(4):
                    gc = qi * 4 + ci
                    for half in range(2):
                        op = ps512.tile([128, 512], F32, tag="ps512")
                        nc.tensor.matmul(
                            op[:], ctxT_sb[:, gc * 128:(gc + 1) * 128],
                            wo_sb[:, half * 512:(half + 1) * 512],
                            start=True, stop=True)
                        if (gc + half) % 2 == 0:
                            nc.scalar.activation(
                                ob[:, ci, half * 512:(half + 1) * 512], op[:],
                                mybir.ActivationFunctionType.Copy)
                        else:
                            nc.vector.tensor_copy(
                                ob[:, ci, half * 512:(half + 1) * 512], op[:])
                t0 = qi * 4 * 128
                nc.sync.dma_start(
                    out_d[t0:t0 + 512, :].rearrange("(c p) e -> p c e", p=128),
                    ob[:])

            # software-pipelined emission; smaller leading slabs so the
            # attention pipeline starts sooner
            widths = [128, 128, 256] + [SLAB] * ((T - 512) // SLAB)
            sc_done = [0] * B
            att_done = 0
            tp_done = 0
            op_done = 0
            proj_chunks = 0
            for wd in widths:
                proj_slab(proj_chunks * 128, wd)
                proj_chunks += wd // 128
                for seq in range(B):
                    while (sc_done[seq] < CPS and
                           seq * CPS + min(sc_done[seq] + 2, CPS - 1)
                           < proj_chunks):
                        for h in range(HPC):
                            scores_j(seq, sc_done[seq], h)
                        sc_done[seq] += 1
                while att_done < NT:
                    seq, c = divmod(att_done, CPS)
                    if min(c + 2, CPS - 1) >= sc_done[seq]:
                        break
                    attention_chunk(att_done)
                    att_done += 1
                    while tp_done < att_done - 2:
                        transpose_chunk(tp_done)
                        tp_done += 1
                while (op_done + 1) * 4 <= tp_done - 6:
                    outproj_quad(op_done)
                    op_done += 1
            while tp_done < NT:
                transpose_chunk(tp_done)
                tp_done += 1
            while op_done * 4 < NT:
                outproj_quad(op_done)
                op_done += 1

    nc.compile()
    return nc


_NC_CACHE = None


def _get_program():
    global _NC_CACHE
    if _NC_CACHE is None:
        _NC_CACHE = _build_program()
    return _NC_CACHE


def make_core_inputs(x, Wqkv, bqkv, Wo):
    """Host-side shard prep: per-core reordered/transposed weight slices.
    bf16 is the on-device compute dtype; casting here (vs on-device) is
    numerically identical and saves a full f32 pass over x.  x is also
    transposed here so the device only ever issues plain DMA copies."""
    import ml_dtypes
    bf16 = ml_dtypes.bfloat16
    xbfT = np.ascontiguousarray(
        np.asarray(x).reshape(T, D).T).astype(bf16)
    in_maps = []
    for ci in range(NCORES):
        heads = [HPC * ci + i for i in range(HPC)]
        rows = []
        brows = []
        for comp in range(3):
            for h in heads:
                sl = slice(h * 3 * HD + comp * HD, h * 3 * HD + (comp + 1) * HD)
                rows.append(Wqkv[sl])
                brows.append(bqkv[sl])
        wq = np.ascontiguousarray(
            np.concatenate(rows, axis=0).T.astype(np.float32)).astype(bf16)
        bq = np.concatenate(brows).astype(np.float32)
        cols = np.concatenate([np.arange(h * HD, (h + 1) * HD) for h in heads])
        woT = np.ascontiguousarray(
            Wo[:, cols].T.astype(np.float32)).astype(bf16)
        in_maps.append({
            "xbfT": xbfT, "wqkvT": wq, "bqkv": bq, "woT": woT,
        })
    return in_maps


def _reference_numpy(x, padding_mask, Wqkv, bqkv, Wo, bo):
    """Exact fallback (only used if padding_mask is not all ones)."""
    NEG = -9e15
    Bx, Sx, Dx = x.shape
    Hh, hd, w = H, HD, W
    qkv = (x.reshape(-1, Dx) @ Wqkv.T + bqkv).reshape(Bx, Sx, Hh, 3, hd)
    q = np.transpose(qkv[..., 0, :], (0, 2, 1, 3))
    k = np.transpose(qkv[..., 1, :], (0, 2, 1, 3))
    v = np.transpose(qkv[..., 2, :], (0, 2, 1, 3))
    nb = Sx // w
    idx = (np.arange(nb) * w)[:, None] + np.arange(3 * w)[None, :]
    kp = np.pad(k, ((0, 0), (0, 0), (w, w), (0, 0)))
    vp = np.pad(v, ((0, 0), (0, 0), (w, w), (0, 0)))
    k_c = kp[:, :, idx, :]
    v_c = vp[:, :, idx, :]
    sc = np.einsum('bhnxd,bhnyd->bhnxy', q.reshape(Bx, Hh, nb, w, hd), k_c)
    x_i = np.arange(w)[:, None]
    j_i = x_i + np.arange(2 * w + 1)[None, :]
    band = sc[..., x_i, j_i]
    key_pos = np.arange(Sx).reshape(nb, w)[:, :, None] - w + np.arange(2 * w + 1)
    valid = (key_pos >= 0) & (key_pos < Sx)
    km = padding_mask[:, np.clip(key_pos, 0, Sx - 1)] != 0
    m = valid[None, None] & km[:, None]
    band = np.where(m, band, NEG)
    band = band / np.sqrt(hd)
    band = band - band.max(axis=-1, keepdims=True)
    e = np.exp(band)
    attn = e / e.sum(axis=-1, keepdims=True)
    attn = np.where(m, attn, 0.0)
    a3 = np.zeros_like(sc)
    a3[..., x_i, j_i] = attn
    ctx = np.einsum('bhnxy,bhnyd->bhnxd', a3, v_c).reshape(Bx, Hh, Sx, hd)
    out = np.transpose(ctx, (0, 2, 1, 3)).reshape(Bx, Sx, Hh * hd)
    return (out @ Wo.T + bo).astype(np.float32)


def kernel(x, padding_mask, Wqkv, bqkv, Wo, bo):
    x = np.asarray(x)
    padding_mask = np.asarray(padding_mask)
    Wqkv = np.asarray(Wqkv, dtype=np.float32)
    bqkv = np.asarray(bqkv, dtype=np.float32)
    Wo = np.asarray(Wo, dtype=np.float32)
    bo = np.asarray(bo, dtype=np.float32)
    if not np.all(padding_mask != 0):
        return _reference_numpy(x.astype(np.float32), padding_mask,
                                Wqkv, bqkv, Wo, bo)
    nc = _get_program()
    in_maps = make_core_inputs(x, Wqkv, bqkv, Wo)
    res = run_bass_kernel_spmd(nc, in_maps, core_ids=list(range(NCORES)))
    acc = np.zeros((T, E), np.float32)
    for ci in range(NCORES):
        acc += np.asarray(res.results[ci]["out_p"]).astype(np.float32)
    # the v bias is not applied on-device; attention rows sum to 1, so
    # ctx = P v0 / den + bv exactly, and its Wo image folds into bo here
    bv = bqkv.reshape(H, 3, HD)[:, 2, :].reshape(E)
    acc += (bo + bv @ Wo.T)[None, :]
    return acc.reshape(B, S, E)


# revision 4
# speedup vs baseline: 1.2838x; 1.2838x over previous
"""Banded (Longformer-style) multi-head attention on 8 TRN2 NeuronCores.

Sharding: 16 heads are split 2-per-core (tensor parallel on H); every
core sees all 8192 tokens.  Compute dtype is bf16 (f32 accumulate in
PSUM); inputs are pre-cast/pre-TRANSPOSED on the host, so the x input
stream is a plain strided DMA copy.

Per-core kernel (single NEFF, fine-grained software-pipelined emission
keyed on the 128-wide key tile index so proj / attention / out-proj
interleave at ~2.5us granularity on the PE):
  1. DMA feature-major xT slabs DRAM->SBUF; project to qT,kT
     (feature-major [d, T]) via w-stationary matmuls and to v TOKEN-major
     via x-stationary matmuls (same FLOPs, no v transpose).  v is stored
     ones-augmented so the P@V matmul also produces the softmax
     denominator.  The v bias is folded into the output bias on the host
     (exact: attention rows sum to 1).
  2. j-major banded attention: for each 128-wide key tile j, one
     scores^T matmul [key,y x query-cols] against the <=5 query chunks
     in its band (K=64), exp on ScalarE without max-subtraction
     (scores are O(+-30), exact in f32), band-corner masking via
     affine_select on the GpSimd engine, then per-query-chunk
     P^T@V_aug accumulation (K=128) and a 1/den fixup on VectorE.
     The Scalar engine runs ONLY the exps so a score tile never queues
     behind unrelated copies (the score matmul's PSUM WAR on the exp is
     the tightest loop in the kernel).
  3. ctx 2-chunk groups transposed feature-major by the Ant DMA-transpose
     unit, dispatched 3+ chunks after their fixup so the SP queue never
     stalls holding the dispatch slot; partial output projection
     ctx_h @ Wo_h.T -> [8192, 1024] bf16.
The host sums the 8 partial outputs and adds the output bias (the
all-reduce step of tensor parallelism, done during the gather).
"""

import sys

sys.path.insert(0, "/opt/trn_rl_repo")

import numpy as np

import concourse.bass as bass
import concourse.mybir as mybir
import concourse.tile as tile
from concourse import bacc
from concourse.bass_utils import run_bass_kernel_spmd

F32 = mybir.dt.float32
BF16 = mybir.dt.bfloat16

B, S, D, E, H, HD = 2, 4096, 1024, 1024, 16, 64
W = 256                    # half window
T = B * S                  # 8192 flattened tokens
NCORES = 8
HPC = H // NCORES          # 2 heads per core
FQKV = 3 * HPC * HD        # 384 projected features per core
NT = T // 128              # 64 token chunks
CPS = S // 128             # 32 chunks per sequence
SLAB = 512                 # proj token slab
VROW = 2 * (HD + 1)        # 130: [v_h0(64) | 1 | v_h1(64) | 1]


def _build_program():
    nc = bacc.Bacc(None, target_bir_lowering=False, debug=False)

    xT_d = nc.dram_tensor("xbfT", [D, T], BF16, kind="ExternalInput")
    wqkvT_d = nc.dram_tensor("wqkvT", [D, FQKV], BF16, kind="ExternalInput")
    bqkv_d = nc.dram_tensor("bqkv", [FQKV], F32, kind="ExternalInput")
    woT_d = nc.dram_tensor("woT", [HPC * HD, E], BF16, kind="ExternalInput")
    out_d = nc.dram_tensor("out_p", [T, E], BF16, kind="ExternalOutput")

    with tile.TileContext(nc) as tc:
        with (
            tc.tile_pool(name="const", bufs=1) as cpool,
            tc.tile_pool(name="big", bufs=1) as bigpool,
            tc.tile_pool(name="xtp", bufs=3) as xtp,
            tc.tile_pool(name="cnp", bufs=4) as cnp,
            tc.tile_pool(name="recp", bufs=4) as recp,
            tc.tile_pool(name="ptp", bufs=10) as ptp,
            tc.tile_pool(name="outsb", bufs=2) as outsb,
            tc.tile_pool(name="ps512", bufs=3, space="PSUM") as ps512,
            tc.tile_pool(name="spsum", bufs=2, space="PSUM") as spsum,
            tc.tile_pool(name="cpsum", bufs=1, space="PSUM") as cpsum,
        ):
            # ---- constants ----
            w_sb = cpool.tile([128, 8, FQKV], BF16, tag="w_sb")
            nc.sync.dma_start(
                w_sb[:], wqkvT_d[:].rearrange("(c p) f -> p c f", p=128))
            b_sb = cpool.tile([128, 3], F32, tag="b_sb")
            nc.sync.dma_start(b_sb[:], bqkv_d[:].rearrange("(a p) -> p a", p=128))
            wo_sb = cpool.tile([128, E], BF16, tag="wo_sb")
            nc.sync.dma_start(wo_sb[:], woT_d[:])

            # ---- persistent activations ----
            q_sb = bigpool.tile([128, T], BF16, tag="q_sb")
            k_sb = bigpool.tile([128, T], BF16, tag="k_sb")
            v_sb = bigpool.tile([128, NT, VROW], BF16, tag="v_sb")
            ctxT_sb = bigpool.tile([128, T], BF16, tag="ctxT_sb")
            # ones columns of the augmented V (cols 64 and 129 of each chunk)
            nc.vector.memset(v_sb[:, :, HD::HD + 1], 1.0)

            # PV accumulators: 2 chunks x 2 heads packed in one PSUM bank
            ctx_ps = cpsum.tile([128, 4, HD + 1], F32, tag="ctx_ps")

            # ---- projection (split into DMA issue and compute) ----
            slabs = ([(0, 128), (128, 128), (256, 256)] +
                     [(512 * k, 512) for k in range(1, T // 512)])
            xT_tiles = {}

            def issue_xT(si):
                t0, wd = slabs[si]
                xT = xtp.tile([128, 8, SLAB], BF16, tag="xT")
                nc.sync.dma_start(
                    xT[:, :, 0:wd],
                    xT_d[:, t0:t0 + wd].rearrange("(c p) t -> p c t", p=128))
                xT_tiles[si] = xT

            def proj_compute(si):
                t0, ntok = slabs[si]
                nck = ntok // 128
                xT = xT_tiles.pop(si)
                # q, k: feature-major [128 feats, ntok]
                for ft in range(2):
                    ps = ps512.tile([128, SLAB], F32, tag="ps512")
                    for c in range(8):
                        nc.tensor.matmul(
                            ps[:, 0:ntok], w_sb[:, c, ft * 128:(ft + 1) * 128],
                            xT[:, c, 0:ntok], start=(c == 0), stop=(c == 7))
                    dest = (q_sb, k_sb)[ft]
                    nc.vector.tensor_scalar_add(
                        dest[:, t0:t0 + ntok], ps[:, 0:ntok], b_sb[:, ft:ft + 1])
                # v: token-major [128 tokens, 128 feats] per chunk (no
                # transpose needed; v bias is folded into bo on the host)
                vps = ps512.tile([128, SLAB], F32, tag="ps512")
                for ck in range(nck):
                    for c in range(8):
                        nc.tensor.matmul(
                            vps[:, ck * 128:(ck + 1) * 128],
                            xT[:, c, ck * 128:(ck + 1) * 128],
                            w_sb[:, c, 2 * 128:3 * 128],
                            start=(c == 0), stop=(c == 7))
                for ck in range(nck):
                    gck = t0 // 128 + ck
                    nc.vector.tensor_copy(
                        v_sb[:, gck, :].rearrange(
                            "p (h r) -> p h r", h=2)[:, :, 0:HD],
                        vps[:, ck * 128:(ck + 1) * 128].rearrange(
                            "p (h r) -> p h r", h=2))

            # j-major scoresT: st_j[y, b*128:(b+1)*128] = k_j^T q_{c}, where
            # c = j-2+b.  pt_j = exp(st_j/8) with band corners zeroed via
            # affine_select on GpSimd.
            pt_tiles = {}
            cn_state = {}

            def scores_j(seq, j, h):
                b_lo = max(0, 2 - j)
                b_hi = min(4, 2 + (CPS - 1) - j)
                gj = seq * CPS + j
                st = spsum.tile([128, 640], F32, tag="st")
                lo, hi = b_lo * 128, (b_hi + 1) * 128
                qcols = (seq * CPS + j - 2) * 128
                pieces = [(a, b) for (a, b) in [(lo, min(hi, 512)), (512, hi)]
                          if b > a]
                for (a, b) in pieces:
                    nc.tensor.matmul(
                        st[:, a:b],
                        k_sb[h * HD:(h + 1) * HD, gj * 128:(gj + 1) * 128],
                        q_sb[h * HD:(h + 1) * HD, qcols + a:qcols + b],
                        start=True, stop=True)
                pt = ptp.tile([128, 640], BF16, tag="pt")
                nc.scalar.activation(
                    pt[:, lo:hi], st[:, lo:hi],
                    mybir.ActivationFunctionType.Exp,
                    scale=float(1.0 / np.sqrt(HD)))
                if b_lo == 0:
                    # b=0 <-> chunk c=j-2, m=4: keep y <= t  (p <= f)
                    nc.gpsimd.affine_select(
                        out=pt[:, 0:128], in_=pt[:, 0:128],
                        compare_op=mybir.AluOpType.is_ge, fill=0.0, base=0,
                        pattern=[[1, 128]], channel_multiplier=-1)
                if b_hi == 4:
                    # b=4 <-> chunk c=j+2, m=0: keep y >= t  (p >= f)
                    nc.gpsimd.affine_select(
                        out=pt[:, 512:640], in_=pt[:, 512:640],
                        compare_op=mybir.AluOpType.is_ge, fill=0.0, base=0,
                        pattern=[[-1, 128]], channel_multiplier=1)
                pt_tiles[(seq, j, h)] = pt

            def attention_chunk(gc):
                seq, c = divmod(gc, CPS)
                qi, ci = divmod(gc, 2)
                m_lo = max(0, 2 - c)
                m_hi = min(4, CPS - 1 - c + 2)
                nm = m_hi - m_lo + 1
                if ci == 0:
                    cn = cnp.tile([128, 2, 2, HD], BF16, tag="cn", name="cn")
                    cn_state[qi] = cn
                cn = cn_state[qi]
                for h in range(HPC):
                    ctx = ctx_ps[:, (gc % 2) * 2 + h, :]
                    for mi, m in enumerate(range(m_lo, m_hi + 1)):
                        j = c - 2 + m
                        pt = pt_tiles[(seq, j, h)]
                        b = c - j + 2
                        nc.tensor.matmul(
                            ctx, pt[:, b * 128:(b + 1) * 128],
                            v_sb[:, seq * CPS + j,
                                 h * (HD + 1):(h + 1) * (HD + 1)],
                            start=(mi == 0), stop=(mi == nm - 1))
                    rec = recp.tile([128, 1], F32, tag="rec")
                    nc.vector.reciprocal(rec[:], ctx[:, HD:HD + 1])
                    nc.vector.tensor_scalar_mul(cn[:, ci, h, :],
                                                ctx[:, 0:HD], rec[:])

            def transpose_pair(pi):
                # 2-chunk batched Ant transpose into feature-major ctxT;
                # dispatched well after the fixup so the SP queue never
                # blocks on it
                nc.sync.dma_start_transpose(
                    ctxT_sb[:, pi * 256:(pi + 1) * 256].rearrange(
                        "p (a b) -> p a b", a=2),
                    cn_state.pop(pi)[:].rearrange("p a b c -> p (a b c)"))

            def outproj_quad(qi):
                ob = outsb.tile([128, 4, E], BF16, tag="ob")
                for ci in range(4):
                    gc = qi * 4 + ci
                    for half in range(2):
                        op = ps512.tile([128, 512], F32, tag="ps512")
                        nc.tensor.matmul(
                            op[:], ctxT_sb[:, gc * 128:(gc + 1) * 128],
                            wo_sb[:, half * 512:(half + 1) * 512],
                            start=True, stop=True)
                        nc.vector.tensor_copy(
                            ob[:, ci, half * 512:(half + 1) * 512], op[:])
                t0 = qi * 4 * 128
                nc.sync.dma_start(
                    out_d[t0:t0 + 512, :].rearrange("(c p) e -> p c e", p=128),
                    ob[:])

            # ---- fine-grained emission keyed on the key-tile index ----
            proj_chunks = 0
            next_slab = 0
            issue_xT(0)

            def ensure_proj(need_chunk):
                nonlocal proj_chunks, next_slab
                while proj_chunks <= need_chunk:
                    if next_slab + 1 < len(slabs):
                        issue_xT(next_slab + 1)
                    proj_compute(next_slab)
                    proj_chunks += slabs[next_slab][1] // 128
                    next_slab += 1

            pairs_done = 0
            op_done = 0

            def drain_outproj(limit_pairs):
                nonlocal op_done
                while (op_done + 1) * 2 <= limit_pairs:
                    outproj_quad(op_done)
                    op_done += 1

            for gj in range(NT):
                seq, j = divmod(gj, CPS)
                ensure_proj(seq * CPS + min(j + 2, CPS - 1))
                for h in range(HPC):
                    scores_j(seq, j, h)
                if gj >= 2:
                    attention_chunk(gj - 2)
                if gj >= 5 and (gj - 5) % 2 == 0:
                    transpose_pair((gj - 5) // 2)
                    pairs_done += 1
                drain_outproj(pairs_done - 2)
            for gc in (NT - 2, NT - 1):
                attention_chunk(gc)
            while pairs_done < NT // 2:
                transpose_pair(pairs_done)
                pairs_done += 1
                drain_outproj(pairs_done - 2)
            drain_outproj(pairs_done)

    nc.compile()
    return nc


_NC_CACHE = None


def _get_program():
    global _NC_CACHE
    if _NC_CACHE is None:
        _NC_CACHE = _build_program()
    return _NC_CACHE


def make_core_inputs(x, Wqkv, bqkv, Wo):
    """Host-side shard prep: per-core reordered/transposed weight slices.
    bf16 is the on-device compute dtype; casting here (vs on-device) is
    numerically identical and saves a full f32 pass over x.  x is also
    transposed here so the device input stream is a plain DMA copy."""
    import ml_dtypes
    bf16 = ml_dtypes.bfloat16
    xbfT = np.ascontiguousarray(
        np.asarray(x).reshape(T, D).T).astype(bf16)
    in_maps = []
    for ci in range(NCORES):
        heads = [HPC * ci + i for i in range(HPC)]
        rows = []
        brows = []
        for comp in range(3):
            for h in heads:
                sl = slice(h * 3 * HD + comp * HD, h * 3 * HD + (comp + 1) * HD)
                rows.append(Wqkv[sl])
                brows.append(bqkv[sl])
        wq = np.ascontiguousarray(
            np.concatenate(rows, axis=0).T.astype(np.float32)).astype(bf16)
        bq = np.concatenate(brows).astype(np.float32)
        cols = np.concatenate([np.arange(h * HD, (h + 1) * HD) for h in heads])
        woT = np.ascontiguousarray(
            Wo[:, cols].T.astype(np.float32)).astype(bf16)
        in_maps.append({
            "xbfT": xbfT, "wqkvT": wq, "bqkv": bq, "woT": woT,
        })
    return in_maps


def _reference_numpy(x, padding_mask, Wqkv, bqkv, Wo, bo):
    """Exact fallback (only used if padding_mask is not all ones)."""
    NEG = -9e15
    Bx, Sx, Dx = x.shape
    Hh, hd, w = H, HD, W
    qkv = (x.reshape(-1, Dx) @ Wqkv.T + bqkv).reshape(Bx, Sx, Hh, 3, hd)
    q = np.transpose(qkv[..., 0, :], (0, 2, 1, 3))
    k = np.transpose(qkv[..., 1, :], (0, 2, 1, 3))
    v = np.transpose(qkv[..., 2, :], (0, 2, 1, 3))
    nb = Sx // w
    idx = (np.arange(nb) * w)[:, None] + np.arange(3 * w)[None, :]
    kp = np.pad(k, ((0, 0), (0, 0), (w, w), (0, 0)))
    vp = np.pad(v, ((0, 0), (0, 0), (w, w), (0, 0)))
    k_c = kp[:, :, idx, :]
    v_c = vp[:, :, idx, :]
    sc = np.einsum('bhnxd,bhnyd->bhnxy', q.reshape(Bx, Hh, nb, w, hd), k_c)
    x_i = np.arange(w)[:, None]
    j_i = x_i + np.arange(2 * w + 1)[None, :]
    band = sc[..., x_i, j_i]
    key_pos = np.arange(Sx).reshape(nb, w)[:, :, None] - w + np.arange(2 * w + 1)
    valid = (key_pos >= 0) & (key_pos < Sx)
    km = padding_mask[:, np.clip(key_pos, 0, Sx - 1)] != 0
    m = valid[None, None] & km[:, None]
    band = np.where(m, band, NEG)
    band = band / np.sqrt(hd)
    band = band - band.max(axis=-1, keepdims=True)
    e = np.exp(band)
    attn = e / e.sum(axis=-1, keepdims=True)
    attn = np.where(m, attn, 0.0)
    a3 = np.zeros_like(sc)
    a3[..., x_i, j_i] = attn
    ctx = np.einsum('bhnxy,bhnyd->bhnxd', a3, v_c).reshape(Bx, Hh, Sx, hd)
    out = np.transpose(ctx, (0, 2, 1, 3)).reshape(Bx, Sx, Hh * hd)
    return (out @ Wo.T + bo).astype(np.float32)


def kernel(x, padding_mask, Wqkv, bqkv, Wo, bo):
    x = np.asarray(x)
    padding_mask = np.asarray(padding_mask)
    Wqkv = np.asarray(Wqkv, dtype=np.float32)
    bqkv = np.asarray(bqkv, dtype=np.float32)
    Wo = np.asarray(Wo, dtype=np.float32)
    bo = np.asarray(bo, dtype=np.float32)
    if not np.all(padding_mask != 0):
        return _reference_numpy(x.astype(np.float32), padding_mask,
                                Wqkv, bqkv, Wo, bo)
    nc = _get_program()
    in_maps = make_core_inputs(x, Wqkv, bqkv, Wo)
    res = run_bass_kernel_spmd(nc, in_maps, core_ids=list(range(NCORES)))
    acc = np.zeros((T, E), np.float32)
    for ci in range(NCORES):
        acc += np.asarray(res.results[ci]["out_p"]).astype(np.float32)
    # the v bias is not applied on-device; attention rows sum to 1, so
    # ctx = P v0 / den + bv exactly, and its Wo image folds into bo here
    bv = bqkv.reshape(H, 3, HD)[:, 2, :].reshape(E)
    acc += (bo + bv @ Wo.T)[None, :]
    return acc.reshape(B, S, E)
